# revision 3
# baseline (speedup 1.0000x reference)
"""MoE layer (8 experts, top-2) on 8 Trainium2 NeuronCores.

Strategy: expert parallelism with host-side dispatch.
  - Host: gate logits (tiny matmul), top-2 + softmax, token->expert dispatch
    with capacity padding, weight/activation pre-layout + cast to bf16.
  - Core e: y_e = relu(x_e @ w1[e].T) @ w2[e].T * gate_scale  (bf16 matmuls,
    fp32 PSUM accumulation), tokens dispatched to expert e only.
  - Host: scatter-add the two expert contributions per token (fp32, exact).
"""

import os

os.environ.setdefault("BASS_NEVER_TRACE", "1")

import numpy as np
import ml_dtypes

D_MODEL = 1024
D_FF = 4096
NUM_EXPERTS = 8
TOP_K = 2
P = 128
C_BLK = 512

BF16 = ml_dtypes.bfloat16

_NC_CACHE: dict[int, object] = {}


def build_moe_nc(C: int):
    """Bass/Tile program for one expert shard with token capacity C.

    DRAM inputs (per core):
      xs  [128, KD, C]        bf16   x_e.T striped: xs[p, k, c] = x_e[c, k*128+p]
      w1s [128, KF, KD, 128]  bf16   w1s[p, fc, k, j] = w1[e][fc*128+j, k*128+p]
      w2s [128, KF, D]        bf16   w2s[p, kf, d]    = w2[e][d, kf*128+p]
      ss  [128, C//128]       f32    ss[p, j] = gate_scale[j*128+p]
    DRAM output:
      y   [C, D] f32          y[c] = gate_scale[c] * relu(x_e[c] @ w1.T) @ w2.T
    """
    import concourse.mybir as mybir
    import concourse.tile as tile
    from concourse import bacc

    D, F = D_MODEL, D_FF
    KD, KF = D // P, F // P  # 8, 32
    bf16, f32 = mybir.dt.bfloat16, mybir.dt.float32
    Relu = mybir.ActivationFunctionType.Relu
    assert C % P == 0

    nc = bacc.Bacc("TRN2", target_bir_lowering=False, debug=False)
    xs = nc.dram_tensor("xs", [P, KD, C], bf16, kind="ExternalInput")
    w1s = nc.dram_tensor("w1s", [P, KF, KD, P], bf16, kind="ExternalInput")
    w2s = nc.dram_tensor("w2s", [P, KF, D], bf16, kind="ExternalInput")
    ss = nc.dram_tensor("ss", [P, C // P], f32, kind="ExternalInput")
    y = nc.dram_tensor("y", [C, D], f32, kind="ExternalOutput")

    blocks = []
    off = 0
    while off < C:
        w = min(C_BLK, C - off)
        blocks.append((off, w))
        off += w

    with tile.TileContext(nc) as tc:
        with (
            tc.tile_pool(name="wpool", bufs=1) as wpool,
            tc.tile_pool(name="xpool", bufs=2) as xpool,
            tc.tile_pool(name="hpool", bufs=1) as hpool,
            tc.tile_pool(name="ypool", bufs=3) as ypool,
            tc.tile_pool(name="phpool", bufs=3, space="PSUM") as phpool,
            tc.tile_pool(name="pypool", bufs=4, space="PSUM") as pypool,
        ):
            s_sb = wpool.tile([P, C // P], f32)
            nc.sync.dma_start(s_sb[:], ss[:])
            # weights as separate chunk tiles so first-block matmuls only
            # wait on their own chunk's DMA, not the full weight load
            WCH = 8
            w1_ch, w2_ch = [], []
            for fc0 in range(0, KF, WCH):
                t = wpool.tile([P, WCH, KD, P], bf16, tag=f"w1_{fc0}")
                nc.sync.dma_start(t[:], w1s[:, fc0 : fc0 + WCH])
                w1_ch.append(t)
            for k0 in range(0, KF, WCH):
                t = wpool.tile([P, WCH, D], bf16, tag=f"w2_{k0}")
                nc.sync.dma_start(t[:], w2s[:, k0 : k0 + WCH])
                w2_ch.append(t)

            def w1_ap(fc, k):
                return w1_ch[fc // WCH][:, fc % WCH, k]

            def w2_ap(k, nsl):
                return w2_ch[k // WCH][:, k % WCH, nsl]

            for off, w in blocks:
                xt = xpool.tile([P, KD, C_BLK], bf16, tag="xt")
                nc.sync.dma_start(xt[:, :, :w], xs[:, :, off : off + w])
                hT = hpool.tile([P, KF, C_BLK], bf16, tag="hT")
                # layer 1: hT[f, c] = relu(sum_d w1T[d, f] * xT[d, c])
                for fc in range(KF):
                    ph = phpool.tile([P, C_BLK], f32, tag="ph")
                    for k in range(KD):
                        nc.tensor.matmul(
                            ph[:, :w],
                            lhsT=w1_ap(fc, k),
                            rhs=xt[:, k, :w],
                            start=(k == 0),
                            stop=(k == KD - 1),
                        )
                    nc.scalar.activation(hT[:, fc, :w], ph[:, :w], Relu)
                # layer 2: y[c, d] = s[c] * sum_f hT[f, c] * w2T[f, d]
                for c0 in range(0, w, P):
                    ys = ypool.tile([P, D], f32, tag="ys")
                    j = (off + c0) // P
                    for ns in range(D // 512):
                        py = pypool.tile([P, 512], f32, tag="py")
                        for k in range(KF):
                            nc.tensor.matmul(
                                py,
                                lhsT=hT[:, k, c0 : c0 + P],
                                rhs=w2_ap(k, slice(ns * 512, (ns + 1) * 512)),
                                start=(k == 0),
                                stop=(k == KF - 1),
                            )
                        nc.scalar.mul(ys[:, ns * 512 : (ns + 1) * 512], py, s_sb[:, j : j + 1])
                    nc.sync.dma_start(y[off + c0 : off + c0 + P, :], ys[:])

    nc.compile()
    return nc


def route_tokens(xf: np.ndarray, gate_w: np.ndarray):
    """Top-2 routing, replicating jax.lax.top_k tie-breaking (lowest index)."""
    logits = xf @ gate_w.astype(np.float32).T  # [T, E]
    top2 = np.argsort(-logits, axis=-1, kind="stable")[:, :TOP_K]
    tv = np.take_along_axis(logits, top2, axis=-1)
    tv = tv - tv.max(axis=-1, keepdims=True)
    ex = np.exp(tv)
    gates = ex / ex.sum(axis=-1, keepdims=True)
    rows, weights = [], []
    for e in range(NUM_EXPERTS):
        r, kpos = np.nonzero(top2 == e)
        rows.append(r)
        weights.append(gates[r, kpos].astype(np.float32))
    return rows, weights


def make_expert_inputs(xf, w1, w2, rows, weights, C):
    """Per-core input arrays in the DRAM layouts build_moe_nc expects."""
    KD, KF = D_MODEL // P, D_FF // P
    in_maps = []
    for e in range(NUM_EXPERTS):
        cnt = len(rows[e])
        X = np.zeros((C, D_MODEL), BF16)
        X[:cnt] = xf[rows[e]].astype(BF16)
        # [C, D] -> [p, k, c]
        xs = np.ascontiguousarray(X.T.reshape(KD, P, C).transpose(1, 0, 2))
        W1 = w1[e].astype(BF16)  # [F, D]
        w1s = np.ascontiguousarray(
            W1.reshape(KF, P, KD, P).transpose(3, 0, 2, 1)
        )  # [p, fc, k, fcol]
        W2 = w2[e].astype(BF16)  # [D, F]
        w2s = np.ascontiguousarray(W2.T.reshape(KF, P, D_MODEL).transpose(1, 0, 2))
        s = np.zeros((C,), np.float32)
        s[:cnt] = weights[e]
        ss = np.ascontiguousarray(s.reshape(C // P, P).T)
        in_maps.append({"xs": xs, "w1s": w1s, "w2s": w2s, "ss": ss})
    return in_maps


def kernel(x, gate_w, w1, w2):
    from concourse.bass_utils import run_bass_kernel_spmd

    x = np.asarray(x)
    gate_w = np.asarray(gate_w)
    w1 = np.asarray(w1)
    w2 = np.asarray(w2)
    B, S, D = x.shape

    xf = x.reshape(-1, D).astype(np.float32)
    rows, weights = route_tokens(xf, gate_w)
    counts = [len(r) for r in rows]
    C = max(C_BLK, -(-max(counts) // P) * P)

    nc = _NC_CACHE.get(C)
    if nc is None:
        nc = _NC_CACHE[C] = build_moe_nc(C)
    in_maps = make_expert_inputs(xf, w1, w2, rows, weights, C)
    res = run_bass_kernel_spmd(nc, in_maps, core_ids=list(range(NUM_EXPERTS)))

    out = np.zeros((B * S, D), np.float32)
    for e in range(NUM_EXPERTS):
        out[rows[e]] += res.results[e]["y"][: counts[e]]
    return out.reshape(B, S, D)
